# revision 40
# baseline (speedup 1.0000x reference)
"""Trainium2 Bass kernel for nn_DifferentiableVCPCBFQP.

Batched tiny-QP (2 vars, m=14 ineq constraints) CBF safety filter:
    min (u - u_nom)^T W (u - u_nom)  s.t.  G(x) u <= h(x)
solved per-sample with a Mehrotra predictor-corrector IPM in fp32, plus an
exact KKT shortcut: where u_nom satisfies G u_nom <= h it is the exact
optimum (lambda = 0 certificate) and is emitted bitwise.

Sharding: pure data parallel, B=32768 split as 4096 samples per core
across 8 NeuronCores. Per-core layout: sample = p*C + c for partition
p in [0,128), column c in [0,C); constraint index m innermost, so most
state lives in (128, C, 14) or paired (128, 2, C, 14) tiles and every
IPM step is an elementwise/segmented-reduce tensor op over all samples.
"""

import math

import numpy as np

import concourse.bacc as bacc
import concourse.bass_isa as bass_isa
import concourse.mybir as mybir
from concourse import tile
from concourse.bass_utils import run_bass_kernel_spmd

# ---------------------------------------------------------------- constants
B = 32768
N_CORES = 8
BPC = B // N_CORES          # 4096 samples per core
P = 128                     # partitions
C = BPC // P                # 32 sample-columns per partition
M = 14                      # constraint rows per sample
N_ITERS = 12                # fp32 IPM iterations (converged ~10, NaN past ~17)

V_MIN, V_MAX = 0.0, 1.0
W_MIN, W_MAX = -2.84, 2.84
ALPHA, DOFF = 1.0, 0.1
ARENA_W, ARENA_H = 10.0, 10.0
ROBOT_R, R_SEP = 0.15, 0.35
XL = ARENA_W - ROBOT_R
YL = ARENA_H - ROBOT_R
PI = math.pi

FP = mybir.dt.float32
AX = mybir.AxisListType
OP = mybir.AluOpType
AF = mybir.ActivationFunctionType

_COMPILED = {}


def build_kernel(n_iters=N_ITERS, debug_tiles=()):
    nc = bacc.Bacc(
        "TRN2", target_bir_lowering=False, debug=False, enable_asserts=False
    )
    d_unom = nc.dram_tensor("u_nom", [BPC, 2], FP, kind="ExternalInput").ap()
    d_states = nc.dram_tensor("states", [BPC, 3], FP, kind="ExternalInput").ap()
    d_opp = nc.dram_tensor("opp", [BPC, 3], FP, kind="ExternalInput").ap()
    d_obs = nc.dram_tensor("obs", [P, 16], FP, kind="ExternalInput").ap()
    d_out = nc.dram_tensor("out", [BPC, 2], FP, kind="ExternalOutput").ap()

    with tile.TileContext(nc) as tc:
        kernel_body(nc, tc, d_unom, d_states, d_opp, d_obs, d_out,
                    n_iters=n_iters, debug_tiles=debug_tiles)

    nc.compile()
    return nc


def kernel_body(nc, tc, d_unom, d_states, d_opp, d_obs, d_out,
                n_iters=N_ITERS, debug_tiles=()):
    V = nc.vector
    GS = nc.gpsimd
    SC = nc.scalar

    def b2(x):   # (P,C,M) -> (P,2,C,M)
        return x.unsqueeze(1).broadcast_to([P, 2, C, M])

    def b3(x):   # (P,C,M) -> (P,3,C,M)
        return x.unsqueeze(1).broadcast_to([P, 3, C, M])

    def bm(x):   # (P,2,C) -> (P,2,C,M)
        return x.unsqueeze(3).broadcast_to([P, 2, C, M])

    def bm1(x):  # (P,C) -> (P,C,M)
        return x.unsqueeze(2).broadcast_to([P, C, M])

    with tc.tile_pool(name="main", bufs=1) as pool:
        # ------------------------------------------------ load inputs
        ST = pool.tile([P, C, 3], FP)
        nc.sync.dma_start(out=ST, in_=d_states.rearrange("(p c) j -> p c j", p=P))
        OPS = pool.tile([P, C, 3], FP)
        nc.scalar.dma_start(out=OPS, in_=d_opp.rearrange("(p c) j -> p c j", p=P))
        UN = pool.tile([P, C, 2], FP)
        nc.sync.dma_start(out=UN, in_=d_unom.rearrange("(p c) j -> p c j", p=P))
        OB = pool.tile([P, 16], FP)
        nc.gpsimd.dma_start(out=OB, in_=d_obs)

        u0n = UN[:, :, 0]
        u1n = UN[:, :, 1]
        x = ST[:, :, 0]
        y = ST[:, :, 1]
        th = ST[:, :, 2]
        xo = OPS[:, :, 0]
        yo = OPS[:, :, 1]
        tho = OPS[:, :, 2]

        # ------------------------------------------------ trig + vcp points
        QX = pool.tile([P, C], FP)
        QY = pool.tile([P, C], FP)
        QXO = pool.tile([P, C], FP)
        QYO = pool.tile([P, C], FP)
        TW4 = pool.tile([P, 4, C], FP)
        SN4 = pool.tile([P, 4, C], FP)

        V.add_range_wrap(TW4[:, 0], th, 0.0, PI, 2.0 * PI)
        V.add_range_wrap(TW4[:, 1], th, PI / 2.0, PI, 2.0 * PI)
        V.add_range_wrap(TW4[:, 2], tho, 0.0, PI, 2.0 * PI)
        V.add_range_wrap(TW4[:, 3], tho, PI / 2.0, PI, 2.0 * PI)
        SC.activation(SN4, TW4, AF.Sin)
        STh = SN4[:, 0]
        CT = SN4[:, 1]
        STo = SN4[:, 2]
        CTo = SN4[:, 3]

        V.affine_then_add(QX, CT, x, DOFF, 0.0)     # qx = x + DOFF*cos
        V.affine_then_add(QY, STh, y, DOFF, 0.0)
        V.affine_then_add(QXO, CTo, xo, DOFF, 0.0)
        V.affine_then_add(QYO, STo, yo, DOFF, 0.0)

        # ------------------------------------------------ G, h
        Gp = pool.tile([P, 2, C, M], FP)   # [G0; G1]
        H = pool.tile([P, C, M], FP)
        G0 = Gp[:, 0]
        G1 = Gp[:, 1]

        # arena rows 0..3 (split ACT/DVE to shorten the serial chain)
        SC.activation(G0[:, :, 0], CT, AF.Copy)
        SC.activation(G0[:, :, 1], CT, AF.Copy, scale=-1.0)
        V.tensor_scalar_mul(G0[:, :, 2], STh, 1.0)
        V.tensor_scalar_mul(G0[:, :, 3], STh, -1.0)
        SC.activation(G1[:, :, 0], STh, AF.Copy, scale=-DOFF)
        SC.activation(G1[:, :, 1], STh, AF.Copy, scale=DOFF)
        V.tensor_scalar_mul(G1[:, :, 2], CT, DOFF)
        V.tensor_scalar_mul(G1[:, :, 3], CT, -DOFF)
        SC.activation(H[:, :, 0], QX, AF.Copy, bias=XL, scale=-1.0)
        SC.activation(H[:, :, 1], QX, AF.Copy, bias=XL)
        V.tensor_scalar(H[:, :, 2], QY, -1.0, YL, op0=OP.mult, op1=OP.add)
        V.tensor_scalar(H[:, :, 3], QY, 1.0, YL, op0=OP.mult, op1=OP.add)

        # obstacle rows 4..8 (K=5), vectorized over obstacles
        K = 5
        ER2 = pool.tile([P, K], FP)   # (r + ROBOT_R)^2
        V.tensor_scalar_add(ER2, OB[:, 10:15], ROBOT_R)
        V.tensor_mul(ER2, ER2, ER2)

        def bK(v):   # (P,C) -> (P,C,K)
            return v.unsqueeze(2).broadcast_to([P, C, K])

        def bKo(v):  # (P,K) -> (P,C,K)
            return v.unsqueeze(1).broadcast_to([P, C, K])

        DX = pool.tile([P, C, K], FP)
        DY = pool.tile([P, C, K], FP)
        TK1 = pool.tile([P, C, K], FP)
        TK2 = pool.tile([P, C, K], FP)
        TK3 = pool.tile([P, C, K], FP)
        TK4 = pool.tile([P, C, K], FP)
        TK5 = pool.tile([P, C, K], FP)
        TK6 = pool.tile([P, C, K], FP)
        V.tensor_sub(DX, bK(QX), bKo(OB[:, 0:5]))
        V.tensor_sub(DY, bK(QY), bKo(OB[:, 5:10]))
        # h_obs = dx^2 + dy^2 - er^2 ; G0 = -2*(dx*ct + dy*st)
        # G1 = 2*DOFF*(dx*st - dy*ct); independent temps so V/GS overlap
        GS.tensor_mul(TK1, DX, DX)
        V.tensor_mul(TK2, DY, DY)
        GS.tensor_mul(TK3, DX, bK(CT))
        V.tensor_mul(TK4, DY, bK(STh))
        GS.tensor_mul(TK5, DX, bK(STh))
        V.tensor_mul(TK6, DY, bK(CT))
        V.tensor_add(TK1, TK1, TK2)
        V.tensor_sub(H[:, :, 4:9], TK1, bKo(ER2))
        V.tensor_add(TK3, TK3, TK4)
        V.tensor_scalar_mul(G0[:, :, 4:9], TK3, -2.0)
        V.tensor_sub(TK5, TK5, TK6)
        V.tensor_scalar_mul(G1[:, :, 4:9], TK5, 2.0 * DOFF)

        # opponent row 9
        DXC = pool.tile([P, C], FP)
        DYC = pool.tile([P, C], FP)
        TC1 = pool.tile([P, C], FP)
        TC2 = pool.tile([P, C], FP)
        TC3 = pool.tile([P, C], FP)
        TC4 = pool.tile([P, C], FP)
        TC5 = pool.tile([P, C], FP)
        TC6 = pool.tile([P, C], FP)
        V.tensor_sub(DXC, QX, QXO)
        V.tensor_sub(DYC, QY, QYO)
        GS.tensor_mul(TC1, DXC, DXC)
        V.tensor_mul(TC2, DYC, DYC)
        GS.tensor_mul(TC3, DXC, CT)
        V.tensor_mul(TC4, DYC, STh)
        GS.tensor_mul(TC5, DXC, STh)
        V.tensor_mul(TC6, DYC, CT)
        V.tensor_add(TC1, TC1, TC2)
        V.tensor_scalar_add(H[:, :, 9], TC1, -float(R_SEP**2))
        V.tensor_add(TC3, TC3, TC4)
        V.tensor_scalar_mul(G0[:, :, 9], TC3, -2.0)
        V.tensor_sub(TC5, TC5, TC6)
        V.tensor_scalar_mul(G1[:, :, 9], TC5, 2.0 * DOFF)

        # control-bound rows 10..13
        V.memset(G0[:, :, 10], -1.0)
        V.memset(G0[:, :, 11], 1.0)
        V.memset(G0[:, :, 12:14], 0.0)
        V.memset(G1[:, :, 10:12], 0.0)
        V.memset(G1[:, :, 12], -1.0)
        V.memset(G1[:, :, 13], 1.0)
        V.memset(H[:, :, 10], -V_MIN)
        V.memset(H[:, :, 11], V_MAX)
        V.memset(H[:, :, 12], -W_MIN)
        V.memset(H[:, :, 13], W_MAX)

        # ------------------------------------------------ derived constants
        P3 = pool.tile([P, 3, C, M], FP)   # [G0*G0, G0*G1, G1*G1]

        u2 = pool.tile([P, 2, C], FP)      # current iterate [u0; u1]
        V.tensor_copy(u2[:, 0], u0n)
        V.tensor_copy(u2[:, 1], u1n)

        # ------------------------------------------------ init s, lam, cert
        SL = pool.tile([P, 2, C, M], FP)       # [s; lam]
        TA = pool.tile([P, 2, C, M], FP)       # scratch pair
        TB = pool.tile([P, 2, C, M], FP)       # scratch pair
        RP = pool.tile([P, C, M], FP)          # r_p
        CMX = pool.tile([P, C], FP)
        MASK = pool.tile([P, C], mybir.dt.uint8)

        # control-bound rows have constant +-1/0 structure: their margins
        # come straight from u_nom (emitted early, off the critical path)
        SC.activation(RP[:, :, 10], u0n, AF.Copy, scale=-1.0)
        SC.activation(RP[:, :, 11], u0n, AF.Copy, bias=-float(V_MAX))
        SC.activation(RP[:, :, 12], u1n, AF.Copy, scale=-1.0, bias=float(W_MIN))
        SC.activation(RP[:, :, 13], u1n, AF.Copy, bias=-float(W_MAX))
        # rows 0..9 carry data-dependent G,h
        bm10 = u2.unsqueeze(3).broadcast_to([P, 2, C, 10])
        V.tensor_mul(TB[:, :, :, 0:10], Gp[:, :, :, 0:10], bm10)
        V.tensor_add(RP[:, :, 0:10], TB[:, 0, :, 0:10], TB[:, 1, :, 0:10])
        V.tensor_sub(RP[:, :, 0:10], RP[:, :, 0:10], H[:, :, 0:10])
        V.reduce_max(CMX, RP, axis=AX.X)
        V.tensor_scalar(MASK, CMX, 0.0, None, op0=OP.is_le)

        # all-core certificate: count violated samples via an idle-PE
        # partition-sum; skip the IPM loop entirely when the count is zero
        IND = pool.tile([P, C], FP)
        ICNT = pool.tile([P, 1], FP)
        ONES = pool.tile([P, 1], FP)
        VCNT = pool.tile([1, 1], FP)
        V.tensor_scalar(IND, CMX, 0.0, None, op0=OP.is_gt)
        V.reduce_sum(ICNT, IND, axis=AX.X)
        V.memset(ONES, 1.0)
        with tc.tile_pool(name="psum", bufs=1, space="PSUM") as psum:
            PCNT = psum.tile([P, 1], FP)
            nc.tensor.matmul(PCNT[:1], ONES, ICNT)
            V.tensor_copy(VCNT, PCNT[:1])

        # ------------------------------------------------ IPM tiles
        SLI = pool.tile([P, 2, C, M], FP)      # [1/s; 1/lam]
        D2 = pool.tile([P, 2, C, M], FP)       # [-ds; -dlam]
        Dg = pool.tile([P, C, M], FP)
        SLAM = pool.tile([P, C, M], FP)
        DGRP = pool.tile([P, C, M], FP)
        VV = pool.tile([P, C, M], FP)
        GDU = pool.tile([P, C, M], FP)
        TD = pool.tile([P, C, M], FP)
        DD = pool.tile([P, C, M], FP)
        RC = pool.tile([P, C, M], FP)
        T1C = pool.tile([P, C, M], FP)
        T3 = pool.tile([P, 3, C, M], FP)

        M3 = pool.tile([P, 3, C], FP)
        G2 = pool.tile([P, 2, C], FP)
        RD2 = pool.tile([P, 2, C], FP)
        R2 = pool.tile([P, 2, C], FP)
        DU2 = pool.tile([P, 2, C], FP)
        QM2 = pool.tile([P, 2, C], FP)
        ADU = pool.tile([P, 2, C], FP)
        MUS = pool.tile([P, C], FP)
        MA = pool.tile([P, C], FP)
        MB = pool.tile([P, C], FP)
        MIA = pool.tile([P, C], FP)
        MIB = pool.tile([P, C], FP)
        MIC = pool.tile([P, C], FP)
        DET = pool.tile([P, C], FP)
        DETI = pool.tile([P, C], FP)
        QM = pool.tile([P, C], FP)
        AF1 = pool.tile([P, C], FP)
        OMA = pool.tile([P, C], FP)
        DDS = pool.tile([P, C], FP)
        MAFF = pool.tile([P, C], FP)
        MUI = pool.tile([P, C], FP)
        RRT = pool.tile([P, C], FP)
        SIMU = pool.tile([P, C], FP)
        TS1 = pool.tile([P, C], FP)
        TS2 = pool.tile([P, C], FP)

        def solve2x2(du2_):
            # du = -M^-1 (rd + g) via premultiplied inverse entries:
            #   du0 = MIC*R1 - MIB*R0 ; du1 = MIC*R0 - MIA*R1
            V.tensor_add(R2, RD2, G2)
            V.tensor_mul(TS1, MIC, R2[:, 1])
            V.tensor_mul(TS2, MIB, R2[:, 0])
            V.tensor_sub(DU2[:, 0], TS1, TS2)
            V.tensor_mul(TS1, MIC, R2[:, 0])
            V.tensor_mul(TS2, MIA, R2[:, 1])
            V.tensor_sub(DU2[:, 1], TS1, TS2)
            # bounded steps: healthy |du| is O(100) max; keeps downstream
            # products finite when det collapsed
            V.tensor_scalar(du2_, du2_, -1e4, 1e4, op0=OP.max, op1=OP.min)

        # Load the global certificate into registers on every engine used in
        # the loop; positive float <=> positive int32 bit pattern, so the
        # branch compares raw bits against 0.
        # skip-path output precomputed before the branch (u2 == u_nom here);
        # the taken branch recomputes it from the final iterate
        OUT = pool.tile([P, C, 2], FP)

        def emit_output():
            V.tensor_scalar(OUT[:, :, 0], u2[:, 0], float(V_MIN), float(V_MAX),
                            op0=OP.max, op1=OP.min)
            V.tensor_scalar(OUT[:, :, 1], u2[:, 1], float(W_MIN), float(W_MAX),
                            op0=OP.max, op1=OP.min)
            V.copy_predicated(OUT[:, :, 0], MASK, u0n)
            V.copy_predicated(OUT[:, :, 1], MASK, u1n)

        emit_output()

        cert_bits = nc.values_load(
            VCNT[0:1, 0:1].bitcast(mybir.dt.int32),
            engines=[mybir.EngineType.DVE, mybir.EngineType.Pool,
                     mybir.EngineType.Activation],
            skip_runtime_bounds_check=True,
        )

        with tc.If(cert_bits > 0, preferred_fallthrough_block=False):
            # loop-only state init (dead on the certified fast path)
            # s = max(h - G u_nom, 1) = max(-margin, 1)
            V.tensor_scalar(SL[:, 0], RP, -1.0, 1.0, op0=OP.mult, op1=OP.max)
            V.memset(SL[:, 1], 1.0)
            # r_p0 = G u_nom + s0 - h = max(margin + 1, 0)
            V.tensor_scalar(RP, RP, 1.0, 0.0, op0=OP.add, op1=OP.max)
            V.tensor_mul(P3[:, 0], G0, G0)
            V.tensor_mul(P3[:, 1], G0, G1)
            V.tensor_mul(P3[:, 2], G1, G1)
            # r_d0 = Q u_nom + p + G^T lam0 = sum_m G  (lam0 = 1, Qu+p = 0)
            V.reduce_sum(RD2, Gp, axis=AX.X)
            for it in range(n_iters):
                # reciprocals of s, lam. No clamp needed: the 0.99 step cap
                # means s,lam >= 0.01^n_iters * init >= 1e-32 > denormals.
                V.reciprocal_approx_accurate(
                    SLI.rearrange("p a c m -> p (a c m)"),
                    SL.rearrange("p a c m -> p (a c m)"),
                    scratch=TA.rearrange("p a c m -> p (a c m)"),
                )
                GS.tensor_mul(Dg, SL[:, 1], SLI[:, 0])
                # guard: keeps M/det finite when mu underflows on samples
                # with active constraints (never binds before convergence)
                V.tensor_scalar_min(Dg, Dg, 1e14)
                GS.tensor_mul(SLAM, SL[:, 0], SL[:, 1])
                V.reduce_sum(MUS, SLAM, axis=AX.X)
                # normal matrix M = Q + sum Dg * G G^T
                GS.tensor_mul(T3, P3, b3(Dg))
                V.reduce_sum(M3, T3, axis=AX.X)
                SC.activation(MA, M3[:, 0], AF.Copy, bias=300.0)
                SC.activation(MB, M3[:, 2], AF.Copy, bias=2.0)
                V.tensor_mul(DET, MA, MB)
                # det >= det(Q) = 600 exactly, but fp32 cancellation can
                # return <=0 when Dg explodes. Floor at a relative fraction
                # of M00*M11 so M^-1 entries stay bounded and pathological
                # samples stall benignly instead of going NaN.
                V.tensor_scalar_mul(TS2, DET, 1e-10)
                V.tensor_mul(TS1, M3[:, 1], M3[:, 1])
                V.tensor_sub(DET, DET, TS1)
                V.tensor_max(DET, DET, TS2)
                V.reciprocal_approx_fast(DETI, DET)
                V.tensor_mul(MIA, MA, DETI)
                V.tensor_mul(MIB, MB, DETI)
                V.tensor_mul(MIC, M3[:, 1], DETI)
                V.tensor_mul(DGRP, Dg, RP)

                # ------------ predictor: rc = s*lam => t1 = rc/s = lam
                V.tensor_sub(VV, DGRP, SL[:, 1])
                GS.tensor_mul(TA, Gp, b2(VV))
                V.reduce_sum(G2, TA, axis=AX.X)
                solve2x2(DU2)
                GS.tensor_mul(TB, Gp, bm(DU2))
                GS.tensor_add(GDU, TB[:, 0], TB[:, 1])
                GS.tensor_add(D2[:, 0], RP, GDU)           # -ds
                V.tensor_mul(TD, Dg, D2[:, 0])
                V.tensor_sub(D2[:, 1], SL[:, 1], TD)       # -dlam
                V.tensor_scalar(D2[:, 1], D2[:, 1], -1e14, 1e14,
                                op0=OP.max, op1=OP.min)
                GS.tensor_mul(TA, D2, SLI)                 # [-ds/s; -dl/lam]
                V.reduce_max(QM2, TA, axis=AX.X)
                V.tensor_max(QM, QM2[:, 0], QM2[:, 1])
                V.tensor_scalar(QM, QM, 1.0, 1e36, op0=OP.max, op1=OP.min)
                V.reciprocal_approx_fast(AF1, QM)          # alpha_aff
                # mu_aff: sum(lam*Dsn + s*Dln) = musum by the complementarity
                # Newton row, so mu_aff_sum = (1-af)*musum + af^2*sum(dd)
                V.tensor_mul(DD, D2[:, 0], D2[:, 1])       # ds*dlam
                V.reduce_sum(DDS, DD, axis=AX.X)
                SC.activation(TS1, AF1, AF.Copy, scale=-1.0, bias=1.0)
                V.tensor_mul(MAFF, TS1, MUS)
                V.tensor_mul(TS2, AF1, DDS)
                V.tensor_mul(TS2, AF1, TS2)
                V.tensor_add(MAFF, MAFF, TS2)
                V.tensor_scalar_max(TS1, MUS, 1e-30)
                V.reciprocal_approx_fast(MUI, TS1)
                V.tensor_mul(RRT, MAFF, MUI)
                # sigma ratio lies in [0,1] in exact arithmetic; clamp so an
                # underflowed mu cannot produce inf^3 * 0 = NaN
                V.tensor_scalar(RRT, RRT, 0.0, 1.0, op0=OP.max, op1=OP.min)
                V.tensor_mul(TS1, RRT, RRT)
                V.tensor_mul(TS1, TS1, RRT)
                V.tensor_mul(TS1, TS1, MUS)
                V.tensor_scalar_mul(SIMU, TS1, 1.0 / M)    # sigma*mu

                # ------------ corrector: rc = s*lam + ds*dlam - sigma*mu
                GS.tensor_add(RC, SLAM, DD)
                V.tensor_sub(RC, RC, bm1(SIMU))
                V.tensor_scalar(RC, RC, -1e6, 1e6, op0=OP.max, op1=OP.min)
                GS.tensor_mul(T1C, RC, SLI[:, 0])          # rc/s
                V.tensor_sub(VV, DGRP, T1C)
                GS.tensor_mul(TA, Gp, b2(VV))
                V.reduce_sum(G2, TA, axis=AX.X)
                solve2x2(DU2)
                GS.tensor_mul(TB, Gp, bm(DU2))
                GS.tensor_add(GDU, TB[:, 0], TB[:, 1])
                GS.tensor_add(D2[:, 0], RP, GDU)
                V.tensor_mul(TD, Dg, D2[:, 0])
                V.tensor_sub(D2[:, 1], T1C, TD)
                V.tensor_scalar(D2[:, 1], D2[:, 1], -1e14, 1e14,
                                op0=OP.max, op1=OP.min)
                GS.tensor_mul(TA, D2, SLI)
                V.reduce_max(QM2, TA, axis=AX.X)
                V.tensor_max(QM, QM2[:, 0], QM2[:, 1])
                V.tensor_scalar(QM, QM, 0.99, 1e36, op0=OP.max, op1=OP.min)
                V.reciprocal_approx_fast(AF1, QM)
                V.tensor_scalar_mul(AF1, AF1, 0.99)        # alpha

                # ------------ updates; residuals contract exactly by (1-a)
                a_bm = AF1.unsqueeze(1).unsqueeze(3).broadcast_to([P, 2, C, M])
                V.tensor_mul(TA, D2, a_bm)
                GS.tensor_sub(SL, SL, TA)
                V.tensor_mul(ADU, DU2, AF1.unsqueeze(1).broadcast_to([P, 2, C]))
                V.tensor_add(u2, u2, ADU)
                if it + 1 < n_iters:
                    SC.activation(OMA, AF1, AF.Copy, scale=-1.0, bias=1.0)
                    V.tensor_mul(RP, RP, bm1(OMA))
                    V.tensor_mul(RD2, RD2,
                                 OMA.unsqueeze(1).broadcast_to([P, 2, C]))

            emit_output()

        # ------------------------------------------------ debug taps
        dbg = dict(Gp=Gp, H=H, SL=SL, CMX=CMX, MASK=MASK, u2=u2, P3=P3,
                   M3=M3, DET=DET, DETI=DETI, SLI=SLI, Dg=Dg, RP=RP,
                   RD2=RD2, DU2=DU2, QM=QM, AF1=AF1, MUS=MUS,
                   SIMU=SIMU, D2=D2, DGRP=DGRP)
        for name in debug_tiles:
            ap = dbg[name]
            d_dbg = nc.dram_tensor(f"dbg_{name}", list(ap.shape), FP,
                                   kind="ExternalOutput").ap()
            nc.sync.dma_start(out=d_dbg, in_=ap)

        # ------------------------------------------------ output
        nc.sync.dma_start(out=d_out.rearrange("(p c) j -> p c j", p=P), in_=OUT)


def make_in_maps(inputs):
    obstacle_xy = np.asarray(inputs["obstacle_xy"], np.float32)
    obstacle_r = np.asarray(inputs["obstacle_r"], np.float32)
    obs_row = np.concatenate(
        [obstacle_xy[:, 0], obstacle_xy[:, 1], obstacle_r, np.zeros(1, np.float32)]
    )  # 16 values, replicated across partitions (pure data movement)
    obs_rep = np.ascontiguousarray(np.tile(obs_row[None, :], (P, 1)))

    u_nominal = np.ascontiguousarray(np.asarray(inputs["u_nominal"], np.float32))
    states = np.ascontiguousarray(np.asarray(inputs["states"], np.float32))
    opp = np.ascontiguousarray(np.asarray(inputs["opponent_states"], np.float32))

    in_maps = []
    for c in range(N_CORES):
        sl = slice(c * BPC, (c + 1) * BPC)
        in_maps.append(
            {
                "u_nom": u_nominal[sl],
                "states": states[sl],
                "opp": opp[sl],
                "obs": obs_rep,
            }
        )
    return in_maps


def kernel(u_nominal, states, obstacle_xy, obstacle_r, opponent_states):
    if "nc" not in _COMPILED:
        _COMPILED["nc"] = build_kernel()
    nc = _COMPILED["nc"]

    in_maps = make_in_maps(
        {
            "u_nominal": u_nominal,
            "states": states,
            "obstacle_xy": obstacle_xy,
            "obstacle_r": obstacle_r,
            "opponent_states": opponent_states,
        }
    )
    res = run_bass_kernel_spmd(nc, in_maps, core_ids=list(range(N_CORES)))
    out = np.concatenate([r["out"] for r in res.results], axis=0)
    return out


# revision 41
# speedup vs baseline: 1.0519x; 1.0519x over previous
"""Trainium2 Bass kernel for nn_DifferentiableVCPCBFQP.

Batched tiny-QP (2 vars, m=14 ineq constraints) CBF safety filter:
    min (u - u_nom)^T W (u - u_nom)  s.t.  G(x) u <= h(x)
solved per-sample with a Mehrotra predictor-corrector IPM in fp32, plus an
exact KKT shortcut: where u_nom satisfies G u_nom <= h it is the exact
optimum (lambda = 0 certificate) and is emitted bitwise.

Sharding: pure data parallel, B=32768 split as 4096 samples per core
across 8 NeuronCores. Per-core layout: sample = p*C + c for partition
p in [0,128), column c in [0,C); constraint index m innermost, so most
state lives in (128, C, 14) or paired (128, 2, C, 14) tiles and every
IPM step is an elementwise/segmented-reduce tensor op over all samples.
"""

import math

import numpy as np

import concourse.bacc as bacc
import concourse.bass_isa as bass_isa
import concourse.mybir as mybir
from concourse import tile
from concourse.bass_utils import run_bass_kernel_spmd

# ---------------------------------------------------------------- constants
B = 32768
N_CORES = 8
BPC = B // N_CORES          # 4096 samples per core
P = 128                     # partitions
C = BPC // P                # 32 sample-columns per partition
M = 14                      # constraint rows per sample
N_ITERS = 12                # fp32 IPM iterations (converged ~10, NaN past ~17)

V_MIN, V_MAX = 0.0, 1.0
W_MIN, W_MAX = -2.84, 2.84
ALPHA, DOFF = 1.0, 0.1
ARENA_W, ARENA_H = 10.0, 10.0
ROBOT_R, R_SEP = 0.15, 0.35
XL = ARENA_W - ROBOT_R
YL = ARENA_H - ROBOT_R
PI = math.pi

FP = mybir.dt.float32
AX = mybir.AxisListType
OP = mybir.AluOpType
AF = mybir.ActivationFunctionType

_COMPILED = {}


def build_kernel(n_iters=N_ITERS, debug_tiles=()):
    nc = bacc.Bacc(
        "TRN2", target_bir_lowering=False, debug=False, enable_asserts=False
    )
    d_unom = nc.dram_tensor("u_nom", [BPC, 2], FP, kind="ExternalInput").ap()
    d_states = nc.dram_tensor("states", [BPC, 3], FP, kind="ExternalInput").ap()
    d_opp = nc.dram_tensor("opp", [BPC, 3], FP, kind="ExternalInput").ap()
    d_obs = nc.dram_tensor("obs", [P, 16], FP, kind="ExternalInput").ap()
    d_out = nc.dram_tensor("out", [BPC, 2], FP, kind="ExternalOutput").ap()

    with tile.TileContext(nc) as tc:
        kernel_body(nc, tc, d_unom, d_states, d_opp, d_obs, d_out,
                    n_iters=n_iters, debug_tiles=debug_tiles)

    nc.compile()
    return nc


def kernel_body(nc, tc, d_unom, d_states, d_opp, d_obs, d_out,
                n_iters=N_ITERS, debug_tiles=()):
    V = nc.vector
    GS = nc.gpsimd
    SC = nc.scalar

    def b2(x):   # (P,C,M) -> (P,2,C,M)
        return x.unsqueeze(1).broadcast_to([P, 2, C, M])

    def b3(x):   # (P,C,M) -> (P,3,C,M)
        return x.unsqueeze(1).broadcast_to([P, 3, C, M])

    def bm(x):   # (P,2,C) -> (P,2,C,M)
        return x.unsqueeze(3).broadcast_to([P, 2, C, M])

    def bm1(x):  # (P,C) -> (P,C,M)
        return x.unsqueeze(2).broadcast_to([P, C, M])

    with tc.tile_pool(name="main", bufs=1) as pool:
        # ------------------------------------------------ load inputs
        ST = pool.tile([P, C, 3], FP)
        nc.sync.dma_start(out=ST, in_=d_states.rearrange("(p c) j -> p c j", p=P))
        OPS = pool.tile([P, C, 3], FP)
        nc.scalar.dma_start(out=OPS, in_=d_opp.rearrange("(p c) j -> p c j", p=P))
        UN = pool.tile([P, C, 2], FP)
        nc.sync.dma_start(out=UN, in_=d_unom.rearrange("(p c) j -> p c j", p=P))
        OB = pool.tile([P, 16], FP)
        nc.gpsimd.dma_start(out=OB, in_=d_obs)

        u0n = UN[:, :, 0]
        u1n = UN[:, :, 1]
        x = ST[:, :, 0]
        y = ST[:, :, 1]
        th = ST[:, :, 2]
        xo = OPS[:, :, 0]
        yo = OPS[:, :, 1]
        tho = OPS[:, :, 2]

        # ------------------------------------------------ trig + vcp points
        QX = pool.tile([P, C], FP)
        QY = pool.tile([P, C], FP)
        QXO = pool.tile([P, C], FP)
        QYO = pool.tile([P, C], FP)
        TW4 = pool.tile([P, 4, C], FP)
        SN4 = pool.tile([P, 4, C], FP)

        V.add_range_wrap(TW4[:, 0], th, 0.0, PI, 2.0 * PI)
        V.add_range_wrap(TW4[:, 1], th, PI / 2.0, PI, 2.0 * PI)
        V.add_range_wrap(TW4[:, 2], tho, 0.0, PI, 2.0 * PI)
        V.add_range_wrap(TW4[:, 3], tho, PI / 2.0, PI, 2.0 * PI)
        SC.activation(SN4, TW4, AF.Sin)
        STh = SN4[:, 0]
        CT = SN4[:, 1]
        STo = SN4[:, 2]
        CTo = SN4[:, 3]

        V.affine_then_add(QX, CT, x, DOFF, 0.0)     # qx = x + DOFF*cos
        V.affine_then_add(QY, STh, y, DOFF, 0.0)
        V.affine_then_add(QXO, CTo, xo, DOFF, 0.0)
        V.affine_then_add(QYO, STo, yo, DOFF, 0.0)

        # ------------------------------------------------ G, h
        Gp = pool.tile([P, 2, C, M], FP)   # [G0; G1]
        H = pool.tile([P, C, M], FP)
        G0 = Gp[:, 0]
        G1 = Gp[:, 1]

        # arena rows 0..3 (split ACT/DVE to shorten the serial chain)
        SC.activation(G0[:, :, 0], CT, AF.Copy)
        SC.activation(G0[:, :, 1], CT, AF.Copy, scale=-1.0)
        V.tensor_scalar_mul(G0[:, :, 2], STh, 1.0)
        V.tensor_scalar_mul(G0[:, :, 3], STh, -1.0)
        SC.activation(G1[:, :, 0], STh, AF.Copy, scale=-DOFF)
        SC.activation(G1[:, :, 1], STh, AF.Copy, scale=DOFF)
        V.tensor_scalar_mul(G1[:, :, 2], CT, DOFF)
        V.tensor_scalar_mul(G1[:, :, 3], CT, -DOFF)
        SC.activation(H[:, :, 0], QX, AF.Copy, bias=XL, scale=-1.0)
        SC.activation(H[:, :, 1], QX, AF.Copy, bias=XL)
        V.tensor_scalar(H[:, :, 2], QY, -1.0, YL, op0=OP.mult, op1=OP.add)
        V.tensor_scalar(H[:, :, 3], QY, 1.0, YL, op0=OP.mult, op1=OP.add)

        # obstacle rows 4..8 (K=5), vectorized over obstacles
        K = 5
        ER2 = pool.tile([P, K], FP)   # (r + ROBOT_R)^2
        V.tensor_scalar_add(ER2, OB[:, 10:15], ROBOT_R)
        V.tensor_mul(ER2, ER2, ER2)

        def bK(v):   # (P,C) -> (P,C,K)
            return v.unsqueeze(2).broadcast_to([P, C, K])

        def bKo(v):  # (P,K) -> (P,C,K)
            return v.unsqueeze(1).broadcast_to([P, C, K])

        DX = pool.tile([P, C, K], FP)
        DY = pool.tile([P, C, K], FP)
        TK1 = pool.tile([P, C, K], FP)
        TK2 = pool.tile([P, C, K], FP)
        TK3 = pool.tile([P, C, K], FP)
        TK4 = pool.tile([P, C, K], FP)
        TK5 = pool.tile([P, C, K], FP)
        TK6 = pool.tile([P, C, K], FP)
        V.tensor_sub(DX, bK(QX), bKo(OB[:, 0:5]))
        V.tensor_sub(DY, bK(QY), bKo(OB[:, 5:10]))
        # h_obs = dx^2 + dy^2 - er^2 ; G0 = -2*(dx*ct + dy*st)
        # G1 = 2*DOFF*(dx*st - dy*ct); independent temps so V/GS overlap
        GS.tensor_mul(TK1, DX, DX)
        V.tensor_mul(TK2, DY, DY)
        GS.tensor_mul(TK3, DX, bK(CT))
        V.tensor_mul(TK4, DY, bK(STh))
        GS.tensor_mul(TK5, DX, bK(STh))
        V.tensor_mul(TK6, DY, bK(CT))
        V.tensor_add(TK1, TK1, TK2)
        V.tensor_sub(H[:, :, 4:9], TK1, bKo(ER2))
        V.tensor_add(TK3, TK3, TK4)
        V.tensor_scalar_mul(G0[:, :, 4:9], TK3, -2.0)
        V.tensor_sub(TK5, TK5, TK6)
        V.tensor_scalar_mul(G1[:, :, 4:9], TK5, 2.0 * DOFF)

        # opponent row 9
        DXC = pool.tile([P, C], FP)
        DYC = pool.tile([P, C], FP)
        TC1 = pool.tile([P, C], FP)
        TC2 = pool.tile([P, C], FP)
        TC3 = pool.tile([P, C], FP)
        TC4 = pool.tile([P, C], FP)
        TC5 = pool.tile([P, C], FP)
        TC6 = pool.tile([P, C], FP)
        V.tensor_sub(DXC, QX, QXO)
        V.tensor_sub(DYC, QY, QYO)
        GS.tensor_mul(TC1, DXC, DXC)
        V.tensor_mul(TC2, DYC, DYC)
        GS.tensor_mul(TC3, DXC, CT)
        V.tensor_mul(TC4, DYC, STh)
        GS.tensor_mul(TC5, DXC, STh)
        V.tensor_mul(TC6, DYC, CT)
        V.tensor_add(TC1, TC1, TC2)
        V.tensor_scalar_add(H[:, :, 9], TC1, -float(R_SEP**2))
        V.tensor_add(TC3, TC3, TC4)
        V.tensor_scalar_mul(G0[:, :, 9], TC3, -2.0)
        V.tensor_sub(TC5, TC5, TC6)
        V.tensor_scalar_mul(G1[:, :, 9], TC5, 2.0 * DOFF)

        # control-bound rows 10..13
        V.memset(G0[:, :, 10], -1.0)
        V.memset(G0[:, :, 11], 1.0)
        V.memset(G0[:, :, 12:14], 0.0)
        V.memset(G1[:, :, 10:12], 0.0)
        V.memset(G1[:, :, 12], -1.0)
        V.memset(G1[:, :, 13], 1.0)
        V.memset(H[:, :, 10], -V_MIN)
        V.memset(H[:, :, 11], V_MAX)
        V.memset(H[:, :, 12], -W_MIN)
        V.memset(H[:, :, 13], W_MAX)

        # ------------------------------------------------ derived constants
        P3 = pool.tile([P, 3, C, M], FP)   # [G0*G0, G0*G1, G1*G1]

        u2 = pool.tile([P, 2, C], FP)      # current iterate [u0; u1]
        V.tensor_copy(u2[:, 0], u0n)
        V.tensor_copy(u2[:, 1], u1n)

        # ------------------------------------------------ init s, lam, cert
        SL = pool.tile([P, 2, C, M], FP)       # [s; lam]
        TA = pool.tile([P, 2, C, M], FP)       # scratch pair
        TB = pool.tile([P, 2, C, M], FP)       # scratch pair
        RP = pool.tile([P, C, M], FP)          # r_p
        CMX = pool.tile([P, C], FP)
        MASK = pool.tile([P, C], mybir.dt.uint8)

        # control-bound rows have constant +-1/0 structure: their margins
        # come straight from u_nom (emitted early, off the critical path)
        SC.activation(RP[:, :, 10], u0n, AF.Copy, scale=-1.0)
        SC.activation(RP[:, :, 11], u0n, AF.Copy, bias=-float(V_MAX))
        SC.activation(RP[:, :, 12], u1n, AF.Copy, scale=-1.0, bias=float(W_MIN))
        SC.activation(RP[:, :, 13], u1n, AF.Copy, bias=-float(W_MAX))
        # rows 0..9 carry data-dependent G,h
        bm10 = u2.unsqueeze(3).broadcast_to([P, 2, C, 10])
        V.tensor_mul(TB[:, :, :, 0:10], Gp[:, :, :, 0:10], bm10)
        V.tensor_add(RP[:, :, 0:10], TB[:, 0, :, 0:10], TB[:, 1, :, 0:10])
        V.tensor_sub(RP[:, :, 0:10], RP[:, :, 0:10], H[:, :, 0:10])
        V.reduce_max(CMX, RP, axis=AX.X)
        V.tensor_scalar(MASK, CMX, 0.0, None, op0=OP.is_le)

        # all-core certificate: count violated samples via an idle-PE
        # partition-sum; skip the IPM loop entirely when the count is zero
        IND = pool.tile([P, C], FP)
        ICNT = pool.tile([P, 1], FP)
        ONES = pool.tile([P, 1], FP)
        VCNT = pool.tile([1, 1], FP)
        V.tensor_scalar(IND, CMX, 0.0, None, op0=OP.is_gt)
        V.reduce_sum(ICNT, IND, axis=AX.X)
        V.memset(ONES, 1.0)
        with tc.tile_pool(name="psum", bufs=1, space="PSUM") as psum:
            PCNT = psum.tile([P, 1], FP)
            nc.tensor.matmul(PCNT[:1], ONES, ICNT)
            V.tensor_copy(VCNT, PCNT[:1])

        # ------------------------------------------------ IPM tiles
        SLI = pool.tile([P, 2, C, M], FP)      # [1/s; 1/lam]
        D2 = pool.tile([P, 2, C, M], FP)       # [-ds; -dlam]
        Dg = pool.tile([P, C, M], FP)
        SLAM = pool.tile([P, C, M], FP)
        DGRP = pool.tile([P, C, M], FP)
        VV = pool.tile([P, C, M], FP)
        GDU = pool.tile([P, C, M], FP)
        TD = pool.tile([P, C, M], FP)
        DD = pool.tile([P, C, M], FP)
        RC = pool.tile([P, C, M], FP)
        T1C = pool.tile([P, C, M], FP)
        T3 = pool.tile([P, 3, C, M], FP)

        M3 = pool.tile([P, 3, C], FP)
        G2 = pool.tile([P, 2, C], FP)
        RD2 = pool.tile([P, 2, C], FP)
        R2 = pool.tile([P, 2, C], FP)
        DU2 = pool.tile([P, 2, C], FP)
        QM2 = pool.tile([P, 2, C], FP)
        ADU = pool.tile([P, 2, C], FP)
        MUS = pool.tile([P, C], FP)
        MA = pool.tile([P, C], FP)
        MB = pool.tile([P, C], FP)
        MIA = pool.tile([P, C], FP)
        MIB = pool.tile([P, C], FP)
        MIC = pool.tile([P, C], FP)
        DET = pool.tile([P, C], FP)
        DETI = pool.tile([P, C], FP)
        QM = pool.tile([P, C], FP)
        AF1 = pool.tile([P, C], FP)
        OMA = pool.tile([P, C], FP)
        DDS = pool.tile([P, C], FP)
        MAFF = pool.tile([P, C], FP)
        MUI = pool.tile([P, C], FP)
        RRT = pool.tile([P, C], FP)
        SIMU = pool.tile([P, C], FP)
        TS1 = pool.tile([P, C], FP)
        TS2 = pool.tile([P, C], FP)

        def solve2x2(du2_):
            # du = -M^-1 (rd + g) via premultiplied inverse entries:
            #   du0 = MIC*R1 - MIB*R0 ; du1 = MIC*R0 - MIA*R1
            V.tensor_add(R2, RD2, G2)
            V.tensor_mul(TS1, MIC, R2[:, 1])
            V.tensor_mul(TS2, MIB, R2[:, 0])
            V.tensor_sub(DU2[:, 0], TS1, TS2)
            V.tensor_mul(TS1, MIC, R2[:, 0])
            V.tensor_mul(TS2, MIA, R2[:, 1])
            V.tensor_sub(DU2[:, 1], TS1, TS2)
            # bounded steps: healthy |du| is O(100) max; keeps downstream
            # products finite when det collapsed
            V.tensor_scalar(du2_, du2_, -1e4, 1e4, op0=OP.max, op1=OP.min)

        # Load the global certificate into registers on every engine used in
        # the loop; positive float <=> positive int32 bit pattern, so the
        # branch compares raw bits against 0.
        # skip-path output precomputed before the branch (u2 == u_nom here);
        # the taken branch recomputes it from the final iterate
        OUT = pool.tile([P, C, 2], FP)

        def emit_output():
            V.tensor_scalar(OUT[:, :, 0], u2[:, 0], float(V_MIN), float(V_MAX),
                            op0=OP.max, op1=OP.min)
            V.tensor_scalar(OUT[:, :, 1], u2[:, 1], float(W_MIN), float(W_MAX),
                            op0=OP.max, op1=OP.min)
            V.copy_predicated(OUT[:, :, 0], MASK, u0n)
            V.copy_predicated(OUT[:, :, 1], MASK, u1n)

        # certified samples satisfy the bound rows, so u_nom is already
        # inside the clip box: the fast-path output is a plain copy (on GS,
        # which is idle here); the taken branch overwrites OUT via
        # emit_output() from the final iterate
        GS.tensor_copy(OUT[:, :, 0], u0n)
        GS.tensor_copy(OUT[:, :, 1], u1n)

        cert_bits = nc.values_load(
            VCNT[0:1, 0:1].bitcast(mybir.dt.int32),
            engines=[mybir.EngineType.DVE, mybir.EngineType.Pool,
                     mybir.EngineType.Activation],
            skip_runtime_bounds_check=True,
        )

        with tc.If(cert_bits > 0, preferred_fallthrough_block=False):
            # loop-only state init (dead on the certified fast path)
            # s = max(h - G u_nom, 1) = max(-margin, 1)
            V.tensor_scalar(SL[:, 0], RP, -1.0, 1.0, op0=OP.mult, op1=OP.max)
            V.memset(SL[:, 1], 1.0)
            # r_p0 = G u_nom + s0 - h = max(margin + 1, 0)
            V.tensor_scalar(RP, RP, 1.0, 0.0, op0=OP.add, op1=OP.max)
            V.tensor_mul(P3[:, 0], G0, G0)
            V.tensor_mul(P3[:, 1], G0, G1)
            V.tensor_mul(P3[:, 2], G1, G1)
            # r_d0 = Q u_nom + p + G^T lam0 = sum_m G  (lam0 = 1, Qu+p = 0)
            V.reduce_sum(RD2, Gp, axis=AX.X)
            for it in range(n_iters):
                # reciprocals of s, lam. No clamp needed: the 0.99 step cap
                # means s,lam >= 0.01^n_iters * init >= 1e-32 > denormals.
                V.reciprocal_approx_accurate(
                    SLI.rearrange("p a c m -> p (a c m)"),
                    SL.rearrange("p a c m -> p (a c m)"),
                    scratch=TA.rearrange("p a c m -> p (a c m)"),
                )
                GS.tensor_mul(Dg, SL[:, 1], SLI[:, 0])
                # guard: keeps M/det finite when mu underflows on samples
                # with active constraints (never binds before convergence)
                V.tensor_scalar_min(Dg, Dg, 1e14)
                GS.tensor_mul(SLAM, SL[:, 0], SL[:, 1])
                V.reduce_sum(MUS, SLAM, axis=AX.X)
                # normal matrix M = Q + sum Dg * G G^T
                GS.tensor_mul(T3, P3, b3(Dg))
                V.reduce_sum(M3, T3, axis=AX.X)
                SC.activation(MA, M3[:, 0], AF.Copy, bias=300.0)
                SC.activation(MB, M3[:, 2], AF.Copy, bias=2.0)
                V.tensor_mul(DET, MA, MB)
                # det >= det(Q) = 600 exactly, but fp32 cancellation can
                # return <=0 when Dg explodes. Floor at a relative fraction
                # of M00*M11 so M^-1 entries stay bounded and pathological
                # samples stall benignly instead of going NaN.
                V.tensor_scalar_mul(TS2, DET, 1e-10)
                V.tensor_mul(TS1, M3[:, 1], M3[:, 1])
                V.tensor_sub(DET, DET, TS1)
                V.tensor_max(DET, DET, TS2)
                V.reciprocal_approx_fast(DETI, DET)
                V.tensor_mul(MIA, MA, DETI)
                V.tensor_mul(MIB, MB, DETI)
                V.tensor_mul(MIC, M3[:, 1], DETI)
                V.tensor_mul(DGRP, Dg, RP)

                # ------------ predictor: rc = s*lam => t1 = rc/s = lam
                V.tensor_sub(VV, DGRP, SL[:, 1])
                GS.tensor_mul(TA, Gp, b2(VV))
                V.reduce_sum(G2, TA, axis=AX.X)
                solve2x2(DU2)
                GS.tensor_mul(TB, Gp, bm(DU2))
                GS.tensor_add(GDU, TB[:, 0], TB[:, 1])
                GS.tensor_add(D2[:, 0], RP, GDU)           # -ds
                V.tensor_mul(TD, Dg, D2[:, 0])
                V.tensor_sub(D2[:, 1], SL[:, 1], TD)       # -dlam
                V.tensor_scalar(D2[:, 1], D2[:, 1], -1e14, 1e14,
                                op0=OP.max, op1=OP.min)
                GS.tensor_mul(TA, D2, SLI)                 # [-ds/s; -dl/lam]
                V.reduce_max(QM2, TA, axis=AX.X)
                V.tensor_max(QM, QM2[:, 0], QM2[:, 1])
                V.tensor_scalar(QM, QM, 1.0, 1e36, op0=OP.max, op1=OP.min)
                V.reciprocal_approx_fast(AF1, QM)          # alpha_aff
                # mu_aff: sum(lam*Dsn + s*Dln) = musum by the complementarity
                # Newton row, so mu_aff_sum = (1-af)*musum + af^2*sum(dd)
                V.tensor_mul(DD, D2[:, 0], D2[:, 1])       # ds*dlam
                V.reduce_sum(DDS, DD, axis=AX.X)
                SC.activation(TS1, AF1, AF.Copy, scale=-1.0, bias=1.0)
                V.tensor_mul(MAFF, TS1, MUS)
                V.tensor_mul(TS2, AF1, DDS)
                V.tensor_mul(TS2, AF1, TS2)
                V.tensor_add(MAFF, MAFF, TS2)
                V.tensor_scalar_max(TS1, MUS, 1e-30)
                V.reciprocal_approx_fast(MUI, TS1)
                V.tensor_mul(RRT, MAFF, MUI)
                # sigma ratio lies in [0,1] in exact arithmetic; clamp so an
                # underflowed mu cannot produce inf^3 * 0 = NaN
                V.tensor_scalar(RRT, RRT, 0.0, 1.0, op0=OP.max, op1=OP.min)
                V.tensor_mul(TS1, RRT, RRT)
                V.tensor_mul(TS1, TS1, RRT)
                V.tensor_mul(TS1, TS1, MUS)
                V.tensor_scalar_mul(SIMU, TS1, 1.0 / M)    # sigma*mu

                # ------------ corrector: rc = s*lam + ds*dlam - sigma*mu
                GS.tensor_add(RC, SLAM, DD)
                V.tensor_sub(RC, RC, bm1(SIMU))
                V.tensor_scalar(RC, RC, -1e6, 1e6, op0=OP.max, op1=OP.min)
                GS.tensor_mul(T1C, RC, SLI[:, 0])          # rc/s
                V.tensor_sub(VV, DGRP, T1C)
                GS.tensor_mul(TA, Gp, b2(VV))
                V.reduce_sum(G2, TA, axis=AX.X)
                solve2x2(DU2)
                GS.tensor_mul(TB, Gp, bm(DU2))
                GS.tensor_add(GDU, TB[:, 0], TB[:, 1])
                GS.tensor_add(D2[:, 0], RP, GDU)
                V.tensor_mul(TD, Dg, D2[:, 0])
                V.tensor_sub(D2[:, 1], T1C, TD)
                V.tensor_scalar(D2[:, 1], D2[:, 1], -1e14, 1e14,
                                op0=OP.max, op1=OP.min)
                GS.tensor_mul(TA, D2, SLI)
                V.reduce_max(QM2, TA, axis=AX.X)
                V.tensor_max(QM, QM2[:, 0], QM2[:, 1])
                V.tensor_scalar(QM, QM, 0.99, 1e36, op0=OP.max, op1=OP.min)
                V.reciprocal_approx_fast(AF1, QM)
                V.tensor_scalar_mul(AF1, AF1, 0.99)        # alpha

                # ------------ updates; residuals contract exactly by (1-a)
                a_bm = AF1.unsqueeze(1).unsqueeze(3).broadcast_to([P, 2, C, M])
                V.tensor_mul(TA, D2, a_bm)
                GS.tensor_sub(SL, SL, TA)
                V.tensor_mul(ADU, DU2, AF1.unsqueeze(1).broadcast_to([P, 2, C]))
                V.tensor_add(u2, u2, ADU)
                if it + 1 < n_iters:
                    SC.activation(OMA, AF1, AF.Copy, scale=-1.0, bias=1.0)
                    V.tensor_mul(RP, RP, bm1(OMA))
                    V.tensor_mul(RD2, RD2,
                                 OMA.unsqueeze(1).broadcast_to([P, 2, C]))

            emit_output()

        # ------------------------------------------------ debug taps
        dbg = dict(Gp=Gp, H=H, SL=SL, CMX=CMX, MASK=MASK, u2=u2, P3=P3,
                   M3=M3, DET=DET, DETI=DETI, SLI=SLI, Dg=Dg, RP=RP,
                   RD2=RD2, DU2=DU2, QM=QM, AF1=AF1, MUS=MUS,
                   SIMU=SIMU, D2=D2, DGRP=DGRP)
        for name in debug_tiles:
            ap = dbg[name]
            d_dbg = nc.dram_tensor(f"dbg_{name}", list(ap.shape), FP,
                                   kind="ExternalOutput").ap()
            nc.sync.dma_start(out=d_dbg, in_=ap)

        # ------------------------------------------------ output
        nc.sync.dma_start(out=d_out.rearrange("(p c) j -> p c j", p=P), in_=OUT)


def make_in_maps(inputs):
    obstacle_xy = np.asarray(inputs["obstacle_xy"], np.float32)
    obstacle_r = np.asarray(inputs["obstacle_r"], np.float32)
    obs_row = np.concatenate(
        [obstacle_xy[:, 0], obstacle_xy[:, 1], obstacle_r, np.zeros(1, np.float32)]
    )  # 16 values, replicated across partitions (pure data movement)
    obs_rep = np.ascontiguousarray(np.tile(obs_row[None, :], (P, 1)))

    u_nominal = np.ascontiguousarray(np.asarray(inputs["u_nominal"], np.float32))
    states = np.ascontiguousarray(np.asarray(inputs["states"], np.float32))
    opp = np.ascontiguousarray(np.asarray(inputs["opponent_states"], np.float32))

    in_maps = []
    for c in range(N_CORES):
        sl = slice(c * BPC, (c + 1) * BPC)
        in_maps.append(
            {
                "u_nom": u_nominal[sl],
                "states": states[sl],
                "opp": opp[sl],
                "obs": obs_rep,
            }
        )
    return in_maps


def kernel(u_nominal, states, obstacle_xy, obstacle_r, opponent_states):
    if "nc" not in _COMPILED:
        _COMPILED["nc"] = build_kernel()
    nc = _COMPILED["nc"]

    in_maps = make_in_maps(
        {
            "u_nominal": u_nominal,
            "states": states,
            "obstacle_xy": obstacle_xy,
            "obstacle_r": obstacle_r,
            "opponent_states": opponent_states,
        }
    )
    res = run_bass_kernel_spmd(nc, in_maps, core_ids=list(range(N_CORES)))
    out = np.concatenate([r["out"] for r in res.results], axis=0)
    return out
